# revision 41
# baseline (speedup 1.0000x reference)
"""Multi-head attention block on 8 TRN2 NeuronCores.

Problem: x[2,2048,768] -> qkv proj -> 12-head attention -> out proj.
Sharding: 24 (batch, head) pairs across 8 cores; core c handles batch
c//4 and heads 3*(c%4)..3*(c%4)+2. Each core computes its heads'
Q,K,V, attention, and a partial output projection; the host sums the
four per-batch partials and adds the bias terms.

Design notes (final):
  - The kernel is ACT-engine-bound: softmax needs 96 exps of
    [128,1024] f32-from-PSUM (~1.1us effective each, ~105us total) and
    the ACT engine is the only one with exp. Everything else is
    scheduled to start that stream early and hide beneath it.
  - All matmul operands are bf16 (1 PE cycle/row at any free size, no
    f32r free<256 penalty); PSUM accumulates f32; DRAM in/out tensors
    are bf16 (host converts), halving DMA bytes. bf16 keeps the final
    Frobenius rel err ~5e-3, well within the 2e-2 gate.
  - Input DMA is split across the sync and gpsimd hardware queues
    (aggregate ~205GB/s), with 1024-wide x^T chunks (2KB partition
    lines) and chunk-packed weights (one fat-line DMA per tensor),
    ordered exactly by consumption: wqk halves on both queues, x^T
    slab0 split 3 chunks/queue, wv, slab1, wp.
  - Phase A, per 1024-column slab: fused Q+K projection (3 PSUM
    groups of 128 rows = 384 rows; wq and q-bias pre-scaled by 0.125
    host-side) + keys-major V (192 free). K-copies on ACT (idle until
    the exps begin), Q (bias-add) and V copies on DVE, all zero-fill
    pads and V ones-columns via cheap gpsimd memsets; the Exp table
    is preloaded via a dummy activation during the DMA wait.
  - Attention: S -> exp -> AV through a global pend queue (lag 2) so
    the PE stays ahead of ACT; softmax denominators come free from a
    ones column appended to V (row 64 of the AV PSUM output);
    normalize = stage the denominator row, approx-reciprocal, gpsimd
    partition-broadcast, one DVE multiply straight out of PSUM.
    PSUM: tag "s" [128,1024] x2 + av0/av1 [65,512] x2 = 8 banks.
  - Output projection: block 0's eight chunks ride the ACT-paced PE
    slack one chunk every other m-round early in block 1 (PSUM via
    the s tag; DVE casts; sync-queue DMA); block 1's chunks form the
    tail with casts alternating ACT/DVE and output DMAs alternating
    sync/gpsimd queues for write bandwidth.
  - k-bias dropped (softmax shift invariance); v-bias and proj-bias
    fold in on the host: out += b_proj + b_v @ w_proj.
  - Known timeline on HW (~173us total): input DMA 3-25us, phase A
    ends ~47us, exp stream ~49-155us (dense, <8us of gaps), tail
    normalize+projection+output-drain ~18us.
"""

import os
import sys

for _p in ("/opt/trn_rl_repo", "/opt/pypackages"):
    if _p not in sys.path:
        sys.path.append(_p)

import numpy as np

B, N, C = 2, 2048, 768
H, D = 12, 64
HPC = 3                    # heads per core
J = HPC * D                # 192 per-core head-dim rows
NCORES = 8
NBLK = 1024                # query-block width (one exp per [128, NBLK])
NB = N // NBLK             # 2
MC = N // 128              # 16 key chunks
KC = C // 128              # 6 contraction chunks for projections

_cache = {}
LAST_RESULTS = None


def _build():
    import concourse.mybir as mybir
    import concourse.tile as tile
    from concourse import bacc

    f32 = mybir.dt.float32
    bf16 = mybir.dt.bfloat16
    Exp = mybir.ActivationFunctionType.Exp
    Copy = mybir.ActivationFunctionType.Copy
    mult = mybir.AluOpType.mult
    add = mybir.AluOpType.add

    nc = bacc.Bacc("TRN2", target_bir_lowering=False, debug=False,
                   num_devices=NCORES)

    # weights arrive chunk-packed along the free axis so each loads in
    # ONE fat-line DMA (2KB+ per partition line)
    xt_d = nc.declare_dram_parameter("xt", [C, N], bf16, isOutput=False)
    wqk_d = nc.declare_dram_parameter("wqk", [128, KC * 2 * J], bf16,
                                      isOutput=False)
    wv_d = nc.declare_dram_parameter("wv", [128, KC * J], bf16,
                                     isOutput=False)
    bq_d = nc.declare_dram_parameter("bq", [J, 1], f32, isOutput=False)
    wp_d = nc.declare_dram_parameter("wp", [128, 2 * C], bf16,
                                     isOutput=False)
    out_d = nc.declare_dram_parameter("out", [N, C], bf16, isOutput=True)

    with tile.TileContext(nc) as tc:
        with (
            tc.tile_pool(name="persist", bufs=1) as pp,
            tc.tile_pool(name="osb", bufs=4) as posb,
            tc.tile_pool(name="etile", bufs=4) as pe,
            tc.tile_pool(name="bcsb", bufs=4) as pbc,
        ):
            bqt = [pp.tile([64, 1], f32, tag=f"bq{h}", name=f"bq{h}")
                   for h in range(HPC)]
            xt = [pp.tile([128, N], bf16, tag=f"xt{i}", name=f"xt{i}")
                  for i in range(KC)]
            wqk_t = pp.tile([128, KC * 2 * J], bf16, tag="wqk",
                            name="wqk")
            wv_t = pp.tile([128, KC * J], bf16, tag="wv", name="wv")
            wqk = [wqk_t[:, 2 * J * i:2 * J * (i + 1)] for i in range(KC)]
            wv = [wv_t[:, J * i:J * (i + 1)] for i in range(KC)]
            # padded Q^T/K^T per head: rows 0:64 data, rows 64:128 zero
            qh = [pp.tile([128, N], bf16, tag=f"qh{h}", name=f"qh{h}")
                  for h in range(HPC)]
            kh = [pp.tile([128, N], bf16, tag=f"kh{h}", name=f"kh{h}")
                  for h in range(HPC)]
            # V keys-major with a ones column per head: [128, 3*65]
            vx = [pp.tile([128, HPC * 65], bf16, tag=f"vx{m}",
                          name=f"vx{m}") for m in range(MC)]
            wp_t = pp.tile([128, 2 * C], bf16, tag="wp", name="wp")
            wp = [wp_t[:, C * t:C * (t + 1)] for t in range(2)]
            # normalized attention outputs: ah2[0] = heads 0,1;
            # ah2[1] rows 0:64 = head 2, rows 64:128 zero
            ah2 = [pp.tile([128, N], bf16, tag=f"ah2{t}", name=f"ah2{t}")
                   for t in range(2)]
            dummy = pp.tile([1, 4], f32, tag="dummy", name="dummy")

            # Dual-queue DMA, 1024-wide x^T chunks (2KB partition lines)
            # split 3 chunks per queue, slab-ordered; weights land in
            # one fat-line DMA each.
            HW = KC * J  # half of the packed wqk columns
            nc.gpsimd.dma_start(wqk_t[:, HW:], wqk_d[:, HW:])
            for i in (3, 4, 5):
                nc.gpsimd.dma_start(xt[i][:, 0:1024],
                                    xt_d[128 * i:128 * (i + 1), 0:1024])
            # wv after slab 0: V isn't consumed until ~8us after the
            # first QK matmul, and ordering it earlier inflates the
            # coalesced slab-0 DMA wait
            nc.gpsimd.dma_start(wv_t[:], wv_d[:, :])
            for i in (3, 4, 5):
                nc.gpsimd.dma_start(xt[i][:, 1024:2048],
                                    xt_d[128 * i:128 * (i + 1), 1024:2048])
            for h in range(HPC):
                nc.sync.dma_start(bqt[h][:], bq_d[64 * h:64 * (h + 1), :])
            nc.sync.dma_start(wqk_t[:, 0:HW], wqk_d[:, 0:HW])
            for s in range(2):
                nsl = slice(1024 * s, 1024 * (s + 1))
                for i in (0, 1, 2):
                    nc.sync.dma_start(xt[i][:, nsl],
                                      xt_d[128 * i:128 * (i + 1), nsl])
            nc.sync.dma_start(wp_t[:], wp_d[:, :])

            # zero-fill pads and ones columns on gpsimd (its memset is
            # far cheaper than DVE's and the engine is otherwise idle);
            # Exp table preload on ACT
            nc.gpsimd.memset(dummy[:], 0.0)
            nc.scalar.activation(dummy[:, 2:4], dummy[:, 0:2], Exp)
            for h in range(HPC):
                nc.gpsimd.memset(qh[h][64:128, :], 0.0)
                nc.gpsimd.memset(kh[h][64:128, :], 0.0)
            nc.gpsimd.memset(ah2[1][64:128, :], 0.0)
            for m in range(MC):
                on = vx[m].rearrange("p (h e) -> p h e", e=65)[:, :, 64:65]
                nc.gpsimd.memset(on, 1.0)

            # ---- Phase A: fused Q+K projection + V, two 1024 slabs ----
            with tc.tile_pool(name="ps1", bufs=2, space="PSUM") as ps1:
                for s in range(2):
                    nsl = slice(1024 * s, 1024 * (s + 1))
                    for g in range(3):
                        gsl = slice(128 * g, 128 * (g + 1))
                        ps = ps1.tile([128, 1024], f32, tag="qk", bufs=3,
                                      name="ps_qk")
                        for half2 in range(2):
                            dst = ps[:, 512 * half2:512 * (half2 + 1)]
                            qsl = slice(1024 * s + 512 * half2,
                                        1024 * s + 512 * (half2 + 1))
                            for k in range(KC):
                                nc.tensor.matmul(
                                    dst, wqk[k][:, gsl], xt[k][:, qsl],
                                    start=(k == 0), stop=(k == KC - 1))
                        for half in range(2):
                            idx = 2 * g + half
                            src = ps[64 * half:64 * (half + 1), :]
                            if idx < HPC:
                                nc.vector.tensor_scalar(
                                    qh[idx][0:64, nsl], src, 1.0,
                                    bqt[idx][:], mult, add)
                            else:
                                nc.scalar.activation(
                                    kh[idx - HPC][0:64, nsl], src, Copy)
                    for m in range(8 * s, 8 * s + 8):
                        msl = slice(128 * m, 128 * (m + 1))
                        psv = ps1.tile([128, J], f32, tag="v", bufs=2,
                                       name="ps_v")
                        for k in range(KC):
                            nc.tensor.matmul(psv[:], xt[k][:, msl], wv[k][:],
                                             start=(k == 0),
                                             stop=(k == KC - 1))
                        vdst = vx[m].rearrange("p (h e) -> p h e",
                                               e=65)[:, :, 0:64]
                        nc.vector.tensor_copy(
                            vdst, psv.rearrange("p (h e) -> p h e", e=64))

            # ---- Phase B: attention + interleaved projection ----
            with tc.tile_pool(name="ps2", bufs=1, space="PSUM") as ps2:

                def s_tile():
                    return ps2.tile([128, NBLK], f32, tag="s", bufs=2,
                                    name="ps_s")

                pend = []

                def flush_one():
                    avh, h, nb, mm, ee = pend.pop(0)
                    vsl = slice(65 * h, 65 * (h + 1))
                    for i in range(NBLK // 512):
                        nc.tensor.matmul(
                            avh[i][:], vx[mm][:, vsl],
                            ee[:, 512 * i:512 * (i + 1)],
                            start=(mm == 0), stop=(mm == MC - 1))
                    if mm != MC - 1:
                        return
                    adst, r0 = ((ah2[0], 0) if h == 0 else
                                (ah2[0], 64) if h == 1 else
                                (ah2[1], 0))
                    # stage both halves first so the copy/recip
                    # (DVE), broadcast (gpsimd) and multiply (DVE)
                    # chains pipeline instead of serializing per half
                    dns, recs, bcss = [], [], []
                    for i in range(NBLK // 512):
                        dn = pbc.tile([1, 512], f32, tag="dn", name="dn")
                        nc.vector.tensor_copy(dn[:], avh[i][64:65, :])
                        dns.append(dn)
                    for i in range(NBLK // 512):
                        rec = pbc.tile([1, 512], f32, tag="rec",
                                       name="rec")
                        nc.vector.reciprocal_approx_fast(rec[:], dns[i][:])
                        recs.append(rec)
                    for i in range(NBLK // 512):
                        bcs = pbc.tile([64, 512], f32, tag="bcs",
                                       name="bcs")
                        nc.gpsimd.partition_broadcast(bcs[:], recs[i][:])
                        bcss.append(bcs)
                    for i in range(NBLK // 512):
                        hf = slice(NBLK * nb + 512 * i,
                                   NBLK * nb + 512 * (i + 1))
                        nc.vector.tensor_mul(
                            adst[r0:r0 + 64, hf], avh[i][0:64, :],
                            bcss[i][:])

                def proj_chunk(mi, tail):
                    msl = slice(128 * mi, 128 * (mi + 1))
                    pj = s_tile()
                    for f0, fn in ((0, 512), (512, 256)):
                        for t in range(2):
                            nc.tensor.matmul(
                                pj[:, f0:f0 + fn], ah2[t][:, msl],
                                wp[t][:, f0:f0 + fn],
                                start=(t == 0), stop=(t == 1))
                    o3 = posb.tile([128, C], bf16, tag="o3", name="o3")
                    # tail casts alternate ACT (idle after the exps) and
                    # DVE; tail DMAs alternate queues for write bandwidth
                    if tail and mi % 2:
                        nc.scalar.activation(o3[:], pj[:, 0:C], Copy)
                    else:
                        nc.vector.tensor_copy(o3[:], pj[:, 0:C])
                    eng = nc.gpsimd if (tail and mi % 2 == 0) else nc.sync
                    eng.dma_start(out_d[msl, :], o3[:])

                for nb in range(NB):
                    for h in range(HPC):
                        avh = [ps2.tile([65, 512], f32, tag=f"av{i}",
                                        bufs=2, name=f"ps_av{i}")
                               for i in range(NBLK // 512)]
                        for m in range(MC):
                            msl = slice(128 * m, 128 * (m + 1))
                            s = s_tile()
                            for i in range(NBLK // 512):
                                nc.tensor.matmul(
                                    s[:, 512 * i:512 * (i + 1)],
                                    kh[h][:, msl],
                                    qh[h][:, NBLK * nb + 512 * i:
                                          NBLK * nb + 512 * (i + 1)])
                            e = pe.tile([128, NBLK], bf16, tag="e",
                                        name="e")
                            nc.scalar.activation(e[:], s[:], Exp)
                            pend.append((avh, h, nb, m, e))
                            if len(pend) > 2:
                                flush_one()
                            # block-0 projection rides the ACT-paced PE
                            # slack early in block 1
                            if nb == 1 and h == 0 and m >= 2 and m % 2 == 0:
                                proj_chunk(m // 2 - 1, False)
                            if nb == 1 and h == 1 and m == 2:
                                proj_chunk(7, False)
                while pend:
                    flush_one()

            # tail projection in its own deep PSUM pool (attention pool
            # closed, all 8 banks free): 4-deep ring makes the tail
            # PE-paced instead of cast-paced
            with tc.tile_pool(name="ps3", bufs=1, space="PSUM") as ps3:
                for mi in range(8, 16):
                    msl = slice(128 * mi, 128 * (mi + 1))
                    pj = ps3.tile([128, C], f32, tag="pj", bufs=4,
                                  name="ps_pj3")
                    for f0, fn in ((0, 512), (512, 256)):
                        for t in range(2):
                            nc.tensor.matmul(
                                pj[:, f0:f0 + fn], ah2[t][:, msl],
                                wp[t][:, f0:f0 + fn],
                                start=(t == 0), stop=(t == 1))
                    o3 = posb.tile([128, C], bf16, tag="o3", name="o3")
                    if mi % 2:
                        nc.scalar.activation(o3[:], pj[:], Copy)
                    else:
                        nc.vector.tensor_copy(o3[:], pj[:])
                    eng = (nc.sync, nc.gpsimd, nc.scalar)[mi % 3]
                    eng.dma_start(out_d[msl, :], o3[:])

    nc.compile()
    return nc


def kernel(x, w_qkv, b_qkv, w_proj, b_proj):
    import ml_dtypes
    from concourse.bass_utils import run_bass_kernel_spmd

    global LAST_RESULTS
    if "nc" not in _cache:
        _cache["nc"] = _build()
    nc = _cache["nc"]

    bf = ml_dtypes.bfloat16
    x = np.asarray(x, dtype=np.float32)
    w_qkv = np.asarray(w_qkv, dtype=np.float32)
    b_qkv = np.asarray(b_qkv, dtype=np.float32)
    w_proj = np.asarray(w_proj, dtype=np.float32)
    b_proj = np.asarray(b_proj, dtype=np.float32)

    in_maps = []
    for c in range(NCORES):
        b = c // 4
        h0 = HPC * (c % 4)
        cs = slice(64 * h0, 64 * (h0 + HPC))
        ks = slice(C + 64 * h0, C + 64 * (h0 + HPC))
        vs = slice(2 * C + 64 * h0, 2 * C + 64 * (h0 + HPC))
        wqk_cat = np.concatenate(
            [w_qkv[:, cs] * 0.125, w_qkv[:, ks]], axis=1)
        wp_pad = np.zeros((2 * 128, C), dtype=np.float32)
        wp_pad[0:128] = w_proj[64 * h0:64 * (h0 + 2), :]
        wp_pad[128:192] = w_proj[64 * (h0 + 2):64 * (h0 + 3), :]

        def chunk_pack(w):
            # [n*128, f] -> [128, n*f]: k-chunks side by side so each
            # weight tensor loads in one fat-line DMA
            n = w.shape[0] // 128
            return np.ascontiguousarray(
                w.reshape(n, 128, -1).transpose(1, 0, 2).reshape(128, -1))

        in_maps.append({
            "xt": np.ascontiguousarray(x[b].T).astype(bf),
            "wqk": chunk_pack(wqk_cat).astype(bf),
            "wv": chunk_pack(w_qkv[:, vs]).astype(bf),
            "bq": np.ascontiguousarray(
                (b_qkv[cs] * 0.125).reshape(J, 1)),
            "wp": chunk_pack(wp_pad).astype(bf),
        })

    res = run_bass_kernel_spmd(nc, in_maps, core_ids=list(range(NCORES)))
    LAST_RESULTS = res

    out = np.zeros((B, N, C), dtype=np.float32)
    for c in range(NCORES):
        out[c // 4] += np.asarray(res.results[c]["out"],
                                  dtype=np.float32)
    out += b_proj + b_qkv[2 * C:] @ w_proj
    return out
